# revision 10
# baseline (speedup 1.0000x reference)
"""Fused DeepFeatureLoss kernel for 8 Trainium2 NeuronCores (v3).

Reference computation (per batch b, N=4096 points, D=32 features):
    pd[i,j] = -||p_i - p_j||^2 / sigma^2          (points, sigma=0.005)
    fd[i,j] = -||f1_i - f2_j||^2
    ce[i]   = -sum_j softmax(pd)[i,j] * log_softmax(fd)[i,j]
    ce_loss[b]  = sum_i ce[i] * w[i]
    reg_loss[b] = mean_{i, c>=3} (f1[i,c]^2 + f2[i,c]^2)

Identity: ce[i] = ln(Zf_i) - S_i/Zp_i with
    Zf_i = sum_j exp(fd[i,j]);  Zp_i = sum_j exp(pd[i,j]);  S_i = sum_j exp(pd)*fd.

Device work per core (batch k//4, rows [1024*(k%4), +1024), 8 blocks of 128):
  - The augmented K=35 matmul writes v = a*fd + b into PSUM with
    a = 184 (bf16-exact; features pre-scaled by sqrt((128/ln2)/184) on the
    host make it an exact Schraudolph constant) and b = 16256 = 127*128.
  - fd chunks live in ONE [128, 3072] PSUM tile as three rotating 1024-col
    regions (region of chunk c of block rb = (4rb+c) % 3).  Per block, the
    ACT engine consumes 3 chunks (one 3072-wide activate over the whole
    tile) or 2 (a 2048-wide activate over two adjacent regions); exp is
    exact via the activation's free affine (scale=1/a, bias=-b/a), written
    back in place, row-summed by the ACT accumulator -> zfa[:, rb].
  - Remaining chunks go the Schraudolph route: DVE tensor_scalar converts
    PSUM fp32 -> int16 SBUF with clamp max(v,0); the int16 bits ARE the
    bf16 exp approximation.  Pool halves the bitcast tile pairwise
    (tensor_tensor add), DVE reduces the 512 remainder -> zfs.  The host
    divides by the calibrated staircase mean KAPPA.
  - Gaussian band (W=384 around the diagonal in Morton order): pd band
    matmul fp32, fd band matmul fp32r; ep = exp(pd) on ACT with accumulator
    -> zp; S via DVE scalar_tensor_tensor accum -> s.
  - Output: 32 partial columns [zfa(8) zfs(16) zp(8) s(8) -> 40] per core.
Host: Zf = zfa + zfs/KAPPA, ce = w*(ln Zf - S/Zp) summed; reg_loss directly
from f1/f2 (O(N) postprocessing of device reductions, like the final
all-reduce the sharding hint assigns to the host).
"""

import math

import ml_dtypes
import numpy as np
from contextlib import ExitStack

import concourse.bacc as bacc
import concourse.bass as bass
import concourse.tile as tile
from concourse import mybir
from concourse.bass_utils import run_bass_kernel_spmd

SIGMA = 0.005
B, N, D = 2, 4096, 32
NCORES = 8
CPB = NCORES // B            # cores per batch = 4
ROWS = N // CPB              # rows per core = 1024
RB = ROWS // 128             # 128-row blocks per core = 8
NFC = 4                      # fd chunks per row block
FCH = N // NFC               # fd chunk width = 1024 (2 PSUM banks)
NREG = 3                     # fd regions in the big PSUM tile
W = 384                      # point-band width
PAD = (W - 128) // 2         # 128
KP = 5                       # augmented K for points
KB = D + 2                   # augmented K for band features = 34
KS = D + 3                   # augmented K for scaled fd = 35
F32 = mybir.dt.float32
F32R = mybir.dt.float32r
BF16 = mybir.dt.bfloat16
I16 = mybir.dt.int16

A_TRUE = 128.0 / math.log(2.0)        # 184.6649652337873
A_USED = 184.0                        # bf16-exact
CSCALE = math.sqrt(A_TRUE / A_USED)   # host feature pre-scale
BCONST = 16256.0                      # 127 * 128, bf16-exact
KAPPA = 1.039720                      # Schraudolph staircase mean (round mode)

# per-block ACT load: 'T' = ACT takes chunks c0..c2 (3072-wide), 'P' = ACT
# takes two adjacent-region chunks (2048-wide).  5T+3P -> 21 ACT chunks.
BLOCK_PLAN = ["T", "P", "T", "P", "T", "P", "T", "T"]
POOL_HALVE = True

_CACHE = {}


def _block_assign(rb, plan):
    """Returns (act_chunks, act_ap_range, dve_chunks) for block rb."""
    r0 = (4 * rb) % NREG
    regions = [(4 * rb + c) % NREG for c in range(NFC)]
    if plan == "T":
        return [0, 1, 2], (0, 3 * FCH), [3]
    # pair: chunks c0,c1 if their regions are ascending-adjacent, else c1,c2
    if r0 <= 1:
        return [0, 1], (r0 * FCH, (r0 + 2) * FCH), [2, 3]
    return [1, 2], (0, 2 * FCH), [0, 3]


def _build():
    nc = bacc.Bacc(trn_type="TRN2")
    afs = nc.declare_dram_parameter("afs", [KS, ROWS], BF16, isOutput=False)
    bfs = nc.declare_dram_parameter("bfs", [KS, N], BF16, isOutput=False)
    apt = nc.declare_dram_parameter("apt", [KP, ROWS], F32, isOutput=False)
    bpt = nc.declare_dram_parameter("bpt", [KP, RB * W], F32, isOutput=False)
    afr = nc.declare_dram_parameter("afr", [KB, ROWS], F32R, isOutput=False)
    bfb = nc.declare_dram_parameter("bfb", [KB, RB * W], F32R, isOutput=False)
    outp = nc.declare_dram_parameter("partials", [128, 40], F32, isOutput=True)

    AF = mybir.ActivationFunctionType
    OP = mybir.AluOpType

    with ExitStack() as ctx:
        tc = ctx.enter_context(tile.TileContext(nc))
        singles = ctx.enter_context(tc.tile_pool(name="singles", bufs=1))
        psum1 = ctx.enter_context(tc.tile_pool(name="psum1", bufs=1, space="PSUM"))
        pdb_pool = ctx.enter_context(tc.tile_pool(name="pdbp", bufs=1, space="PSUM"))
        fdb_pool = ctx.enter_context(tc.tile_pool(name="fdbp", bufs=1, space="PSUM"))
        e16_pool = ctx.enter_context(tc.tile_pool(name="e16p", bufs=2))
        h_pool = ctx.enter_context(tc.tile_pool(name="hp", bufs=2))
        scr_pool = ctx.enter_context(tc.tile_pool(name="scrp", bufs=2))
        ep_pool = ctx.enter_context(tc.tile_pool(name="epp", bufs=2))
        ss_pool = ctx.enter_context(tc.tile_pool(name="ssp", bufs=2))

        # ---- input loads: critical operands first, split so the first
        # matmul's gate is small; spread across SP and ACT HWDGE queues ----
        afs_sb = singles.tile([128, ROWS], BF16)
        bfs_sb = singles.tile([128, N], BF16)
        nc.sync.dma_start(out=afs_sb[0:KS, :], in_=afs[:, :])
        nc.sync.dma_start(out=bfs_sb[0:KS, 0:FCH], in_=bfs[:, 0:FCH])
        nc.sync.dma_start(out=bfs_sb[0:KS, FCH:N], in_=bfs[:, FCH:N])
        nc.scalar.dma_start(out=afs_sb[64 : 64 + KS, :], in_=afs[:, :])
        nc.scalar.dma_start(out=bfs_sb[64 : 64 + KS, 0:FCH], in_=bfs[:, 0:FCH])
        nc.scalar.dma_start(out=bfs_sb[64 : 64 + KS, FCH:N], in_=bfs[:, FCH:N])
        # Pool SWDGE: band operands (cheap queue issue, needed ~10us in)
        apt_sb = singles.tile([128, ROWS], F32)
        bpt_sb = singles.tile([128, RB * W], F32)
        afr_sb = singles.tile([KB, ROWS], F32R)
        bfb_sb = singles.tile([KB, RB * W], F32R)
        nc.gpsimd.dma_start(out=apt_sb[96 : 96 + KP, :], in_=apt[:, :])
        nc.gpsimd.dma_start(out=bpt_sb[96 : 96 + KP, :], in_=bpt[:, :])
        nc.gpsimd.dma_start(out=apt_sb[64 : 64 + KP, :], in_=apt[:, :])
        nc.gpsimd.dma_start(out=bpt_sb[64 : 64 + KP, :], in_=bpt[:, :])
        nc.gpsimd.dma_start(out=afr_sb[:, :], in_=afr[:, :])
        nc.gpsimd.dma_start(out=bfb_sb[:, :], in_=bfb[:, :])

        bias_sb = singles.tile([128, 1], F32)
        nc.vector.memset(bias_sb, -BCONST / A_TRUE)

        out_sb = singles.tile([128, 40], F32)
        nc.vector.memset(out_sb, 0.0)
        zfa = out_sb[:, 0:8]
        zfs = out_sb[:, 8:24]
        zp = out_sb[:, 24:32]
        s_ = out_sb[:, 32:40]

        fd_big = psum1.tile([128, NREG * FCH], F32)

        for rb in range(RB):
            r0 = rb * 128
            act_chunks, (a_lo, a_hi), dve_chunks = _block_assign(rb, BLOCK_PLAN[rb])
            for c in range(NFC):
                g = (4 * rb + c) % NREG
                reg = fd_big[:, g * FCH : (g + 1) * FCH]
                j0 = c * FCH
                nc.tensor.matmul(
                    reg[:, 0:512],
                    lhsT=afs_sb[0:KS, r0 : r0 + 128],
                    rhs=bfs_sb[0:KS, j0 : j0 + 512],
                    start=True,
                    stop=True,
                )
                nc.tensor.matmul(
                    reg[:, 512:1024],
                    lhsT=afs_sb[64 : 64 + KS, r0 : r0 + 128],
                    rhs=bfs_sb[64 : 64 + KS, j0 + 512 : j0 + 1024],
                    start=True,
                    stop=True,
                    tile_position=(64, 0),
                )
                if c == max(act_chunks):
                    # all ACT chunks of this block are in PSUM: one wide exp
                    nc.scalar.activation(
                        out=fd_big[:, a_lo:a_hi],
                        in_=fd_big[:, a_lo:a_hi],
                        func=AF.Exp,
                        scale=1.0 / A_TRUE,
                        bias=bias_sb[:, 0:1],
                        accum_out=zfa[:, rb : rb + 1],
                    )
                if c in dve_chunks:
                    di = dve_chunks.index(c)
                    e16 = e16_pool.tile([128, FCH], I16, tag="e16")
                    nc.vector.tensor_scalar(
                        out=e16[:, :],
                        in0=reg[:, :],
                        scalar1=0.0,
                        scalar2=None,
                        op0=OP.max,
                    )
                    ebf = e16[:, :].bitcast(BF16)
                    zcol = zfs[:, rb * 2 + di : rb * 2 + di + 1]
                    if POOL_HALVE:
                        half = h_pool.tile([128, 512], BF16, tag="half")
                        nc.gpsimd.tensor_tensor(
                            out=half[:, :],
                            in0=e16[:, 0:512].bitcast(BF16),
                            in1=e16[:, 512:1024].bitcast(BF16),
                            op=OP.add,
                        )
                        scr = scr_pool.tile([128, 512], BF16, tag="scr")
                        nc.vector.tensor_scalar(
                            out=scr[:, :],
                            in0=half[:, :],
                            scalar1=1.0,
                            scalar2=0.0,
                            op0=OP.mult,
                            op1=OP.add,
                            accum_out=zcol,
                        )
                    else:
                        scr = scr_pool.tile([128, FCH], BF16, tag="scr")
                        nc.vector.tensor_scalar(
                            out=scr[:, :],
                            in0=ebf,
                            scalar1=1.0,
                            scalar2=0.0,
                            op0=OP.mult,
                            op1=OP.add,
                            accum_out=zcol,
                        )
            # ---- band ----
            fdbt = fdb_pool.tile([128, W], F32, tag="fdbt", name=f"fdb_{rb}")
            nc.tensor.matmul(
                fdbt[:, :],
                lhsT=afr_sb[0:KB, r0 : r0 + 128],
                rhs=bfb_sb[0:KB, rb * W : (rb + 1) * W],
                start=True,
                stop=True,
            )
            pb = 96 if rb % 2 == 0 else 64
            pdbt = pdb_pool.tile([128, W], F32, tag="pdbt", name=f"pdb_{rb}")
            nc.tensor.matmul(
                pdbt[:, :],
                lhsT=apt_sb[pb : pb + KP, r0 : r0 + 128],
                rhs=bpt_sb[pb : pb + KP, rb * W : (rb + 1) * W],
                start=True,
                stop=True,
                tile_position=(pb, 0),
            )
            ep = ep_pool.tile([128, W], BF16, tag="ep")
            nc.scalar.activation(
                out=ep, in_=pdbt[:, :], func=AF.Exp, accum_out=zp[:, rb : rb + 1]
            )
            sscr = ss_pool.tile([128, W], BF16, tag="sscr")
            nc.vector.scalar_tensor_tensor(
                out=sscr,
                in0=fdbt[:, :],
                scalar=1.0,
                in1=ep[:, :],
                op0=OP.mult,
                op1=OP.mult,
                accum_out=s_[:, rb : rb + 1],
            )

        nc.sync.dma_start(out=outp[:, :], in_=out_sb[:, :])
    return nc


def _morton(p, bits=10):
    q = np.minimum((p * (1 << bits)).astype(np.uint64), (1 << bits) - 1)
    code = np.zeros(len(p), np.uint64)
    for b in range(bits):
        for dim in range(3):
            code |= ((q[:, dim] >> np.uint64(b)) & np.uint64(1)) << np.uint64(3 * b + dim)
    return code


def _fp22(x):
    return (x.view(np.uint32) & np.uint32(0xFFFFFC00)).view(np.float32)


def _prep_batch(b, points, pointfea1, pointfea2, weights):
    perm = np.argsort(_morton(points[b]))
    inv = np.float32(1.0 / (SIGMA * SIGMA))
    p = points[b][perm]
    f1 = pointfea1[b][perm]
    f2 = pointfea2[b][perm]

    p2 = (p * p).sum(1)
    onesN = np.ones((N, 1), np.float32)

    a_pts = np.concatenate([2.0 * p * inv, onesN, (p2 * inv)[:, None]], 1).astype(np.float32)
    b_pts = np.concatenate([p, -(p2 * inv)[:, None], -onesN], 1).astype(np.float32)

    f1sq = (f1 * f1).sum(1)
    f2sq = (f2 * f2).sum(1)
    a_fea = _fp22(np.concatenate([2.0 * f1, onesN, f1sq[:, None]], 1).astype(np.float32))
    b_fea = _fp22(np.concatenate([f2, -f2sq[:, None], -onesN], 1).astype(np.float32))

    c = np.float32(CSCALE)
    f1c = c * f1
    f2c = c * f2
    f1csq = (f1c * f1c).sum(1)
    f2csq = (f2c * f2c).sum(1)
    au = np.float32(A_USED)
    a_s = np.concatenate(
        [2.0 * au * f1c, au * onesN, (au * f1csq)[:, None], onesN], 1
    ).astype(ml_dtypes.bfloat16)
    b_s = np.concatenate(
        [f2c, -f2csq[:, None], -onesN, np.float32(BCONST) * onesN], 1
    ).astype(ml_dtypes.bfloat16)
    return a_pts, b_pts, a_fea, b_fea, a_s, b_s


def make_in_maps(points, pointfea1, pointfea2, weights):
    points = np.asarray(points, np.float32)
    pointfea1 = np.asarray(pointfea1, np.float32)
    pointfea2 = np.asarray(pointfea2, np.float32)
    weights = np.asarray(weights, np.float32)

    batch_data = [
        _prep_batch(b, points, pointfea1, pointfea2, weights) for b in range(B)
    ]
    in_maps = []
    for k in range(NCORES):
        b = k // CPB
        r0 = (k % CPB) * ROWS
        a_pts, b_pts, a_fea, b_fea, a_s, b_s = batch_data[b]
        bpt_band = np.empty((KP, RB * W), np.float32)
        bfb_band = np.empty((KB, RB * W), np.float32)
        for rb in range(RB):
            g0 = r0 + rb * 128
            s = min(max(g0 - PAD, 0), N - W)
            bpt_band[:, rb * W : (rb + 1) * W] = b_pts[s : s + W].T
            bfb_band[:, rb * W : (rb + 1) * W] = b_fea[s : s + W].T
        in_maps.append(
            {
                "afs": np.ascontiguousarray(a_s[r0 : r0 + ROWS].T),
                "bfs": np.ascontiguousarray(b_s.T),
                "apt": np.ascontiguousarray(a_pts[r0 : r0 + ROWS].T),
                "bpt": bpt_band,
                "afr": np.ascontiguousarray(a_fea[r0 : r0 + ROWS].T),
                "bfb": bfb_band,
            }
        )
    return in_maps


def get_nc():
    if "nc" not in _CACHE:
        nc = _build()
        nc.finalize()
        _CACHE["nc"] = nc
    return _CACHE["nc"]


def combine_partials(parts, points, pointfea1, pointfea2, weights):
    """parts: [NCORES, 128, 40]. Host: Zf assembly, ln, ce sum, reg."""
    parts = np.asarray(parts, np.float64)
    weights = np.asarray(weights, np.float32)
    ce = np.zeros(B, np.float64)
    for k in range(NCORES):
        b = k // CPB
        r0 = (k % CPB) * ROWS
        pp = parts[k]
        zf = pp[:, 0:8] + pp[:, 8:24].reshape(128, 8, 2).sum(2) / KAPPA
        zp = pp[:, 24:32]
        s = pp[:, 32:40]
        ce_rows = np.log(zf) - s / zp          # [128 part, 8 blocks]
        perm = _CACHE[f"perm{b}"]
        w = weights[b, :, 0][perm][r0 : r0 + ROWS].reshape(8, 128)  # [rb, p]
        ce[b] += (ce_rows.T * w).sum()
    f1 = np.asarray(pointfea1, np.float64)
    f2 = np.asarray(pointfea2, np.float64)
    reg = (f1[:, :, 3:] ** 2 + f2[:, :, 3:] ** 2).mean(2).mean(1)
    return ce.astype(np.float32), reg.astype(np.float32)


def kernel(points, pointfea1, pointfea2, weights):
    nc = get_nc()
    points = np.asarray(points, np.float32)
    for b in range(B):
        _CACHE[f"perm{b}"] = np.argsort(_morton(points[b]))
    in_maps = make_in_maps(points, pointfea1, pointfea2, weights)
    res = run_bass_kernel_spmd(nc, in_maps, core_ids=list(range(NCORES)))
    parts = np.stack([res.results[k]["partials"] for k in range(NCORES)])
    return combine_partials(parts, points, pointfea1, pointfea2, weights)


# revision 11
# speedup vs baseline: 1.1053x; 1.1053x over previous
"""Fused DeepFeatureLoss kernel for 8 Trainium2 NeuronCores (v4).

Reference computation (per batch b, N=4096 points, D=32 features):
    pd[i,j] = -||p_i - p_j||^2 / sigma^2          (points, sigma=0.005)
    fd[i,j] = -||f1_i - f2_j||^2
    ce[i]   = -sum_j softmax(pd)[i,j] * log_softmax(fd)[i,j]
    ce_loss[b]  = sum_i ce[i] * w[i]
    reg_loss[b] = mean_{i, c>=3} (f1[i,c]^2 + f2[i,c]^2)

Identity: ce[i] = ln(Zf_i) - S_i/Zp_i with
    Zf_i = sum_j exp(fd[i,j]);  Zp_i = sum_j exp(pd[i,j]);  S_i = sum_j exp(pd)*fd.

Device work per core (batch k//4, rows [1024*(k%4), +1024), 8 blocks of 128):
  - The augmented K=35 bf16 matmul writes v = a*fd + b into PSUM chunks of
    1024 cols (pool of 3), a = 184 (bf16-exact; features pre-scaled by
    sqrt((128/ln2)/184) host-side so a is an exact Schraudolph constant),
    b = 16256 = 127*128.
  - ACT chunks (2-3 per block): exact exp via the activation's free affine
    (scale=1/a, bias=-b/a), in place, row-summed by the ACT accumulator.
  - DVE chunks: Schraudolph exp - tensor_scalar converts PSUM fp32 ->
    int16 SBUF with clamp max(v,0); the int16 bits ARE the bf16 exp
    approximation.  Pool folds the bitcast tile in half (tensor_tensor
    add), DVE reduces the 512 remainder.  Host divides by the staircase
    mean KAPPA.
  - Gaussian band (W=384 around the diagonal in Morton order, exact
    permutation): pd band matmul in fp32r over per-block RECENTERED fp22
    points (recentering shrinks |p|^2/sigma^2 so the lost mantissa bits
    stay below the softmax noise floor - validated 6e-5..1e-4 across
    seeds); fd band matmul fp32r.  ep = exp(pd) on ACT with accumulator
    -> zp; S via DVE scalar_tensor_tensor accum -> s.
Host: Zf = zfa + zfs/KAPPA, ce = w*(ln Zf - S/Zp) summed; reg_loss directly
from f1/f2 (O(N) postprocessing, like the final all-reduce the sharding
hint assigns to the host).
"""

import math

import ml_dtypes
import numpy as np
from contextlib import ExitStack

import concourse.bacc as bacc
import concourse.bass as bass
import concourse.tile as tile
from concourse import mybir
from concourse.bass_utils import run_bass_kernel_spmd

SIGMA = 0.005
B, N, D = 2, 4096, 32
NCORES = 8
CPB = NCORES // B            # cores per batch = 4
ROWS = N // CPB              # rows per core = 1024
RB = ROWS // 128             # 128-row blocks per core = 8
NFC = 4                      # fd chunks per row block
FCH = N // NFC               # fd chunk width = 1024 (2 PSUM banks)
W = 384                      # point-band width
PAD = (W - 128) // 2         # 128
KP = 5                       # augmented K for points
KB = D + 2                   # augmented K for band features = 34
KS = D + 3                   # augmented K for scaled fd = 35
F32 = mybir.dt.float32
F32R = mybir.dt.float32r
BF16 = mybir.dt.bfloat16
I16 = mybir.dt.int16

A_TRUE = 128.0 / math.log(2.0)        # 184.6649652337873
A_USED = 184.0                        # bf16-exact
CSCALE = math.sqrt(A_TRUE / A_USED)   # host feature pre-scale
BCONST = 16256.0                      # 127 * 128, bf16-exact
KAPPA = 1.039720                      # Schraudolph staircase mean (round mode)

# chunks per block handled by ACT (exact exp + accumulator); rest by DVE.
ACT_PER_BLOCK = [3, 2, 3, 2, 2, 3, 2, 2]          # sum = 19
POOL_HALVE = True

_CACHE = {}


def _build():
    nc = bacc.Bacc(trn_type="TRN2")
    # ab = [A-rows (1024) | B-cols (4096)] merged so one DMA per row group
    # covers the first matmul's gate.
    ab = nc.declare_dram_parameter("ab", [KS, ROWS + N], BF16, isOutput=False)
    apt = nc.declare_dram_parameter("apt", [KP, ROWS], F32R, isOutput=False)
    bpt = nc.declare_dram_parameter("bpt", [KP, RB * W], F32R, isOutput=False)
    afr = nc.declare_dram_parameter("afr", [KB, ROWS], F32R, isOutput=False)
    bfb = nc.declare_dram_parameter("bfb", [KB, RB * W], F32R, isOutput=False)
    outp = nc.declare_dram_parameter("partials", [128, 56], F32, isOutput=True)

    AF = mybir.ActivationFunctionType
    OP = mybir.AluOpType

    with ExitStack() as ctx:
        tc = ctx.enter_context(tile.TileContext(nc))
        singles = ctx.enter_context(tc.tile_pool(name="singles", bufs=1))
        fd_pool = ctx.enter_context(tc.tile_pool(name="fdp", bufs=3, space="PSUM"))
        pdb_pool = ctx.enter_context(tc.tile_pool(name="pdbp", bufs=1, space="PSUM"))
        fdb_pool = ctx.enter_context(tc.tile_pool(name="fdbp", bufs=1, space="PSUM"))
        e16_pool = ctx.enter_context(tc.tile_pool(name="e16p", bufs=2))
        h_pool = ctx.enter_context(tc.tile_pool(name="hp", bufs=2))
        scr_pool = ctx.enter_context(tc.tile_pool(name="scrp", bufs=2))
        ep_pool = ctx.enter_context(tc.tile_pool(name="epp", bufs=2))
        ss_pool = ctx.enter_context(tc.tile_pool(name="ssp", bufs=2))

        # ---- input loads ----
        ab_sb = singles.tile([128, ROWS + N], BF16)
        nc.sync.dma_start(out=ab_sb[0:KS, 0:2048], in_=ab[:, 0:2048])
        nc.sync.dma_start(out=ab_sb[0:KS, 2048:5120], in_=ab[:, 2048:5120])
        nc.scalar.dma_start(out=ab_sb[64 : 64 + KS, 0:2048], in_=ab[:, 0:2048])
        nc.scalar.dma_start(out=ab_sb[64 : 64 + KS, 2048:5120], in_=ab[:, 2048:5120])
        afs_sb = ab_sb[:, 0:ROWS]
        bfs_sb = ab_sb[:, ROWS : ROWS + N]
        # Pool SWDGE: band operands (cheap queue issue, needed ~12us in)
        apt_sb = singles.tile([128, ROWS], F32R)
        bpt_sb = singles.tile([128, RB * W], F32R)
        afr_sb = singles.tile([KB, ROWS], F32R)
        bfb_sb = singles.tile([KB, RB * W], F32R)
        nc.gpsimd.dma_start(out=apt_sb[96 : 96 + KP, :], in_=apt[:, :])
        nc.gpsimd.dma_start(out=bpt_sb[96 : 96 + KP, :], in_=bpt[:, :])
        nc.gpsimd.dma_start(out=apt_sb[64 : 64 + KP, :], in_=apt[:, :])
        nc.gpsimd.dma_start(out=bpt_sb[64 : 64 + KP, :], in_=bpt[:, :])
        nc.gpsimd.dma_start(out=afr_sb[:, :], in_=afr[:, :])
        nc.gpsimd.dma_start(out=bfb_sb[:, :], in_=bfb[:, :])

        bias_sb = singles.tile([128, 1], F32)
        nc.vector.memset(bias_sb, -BCONST / A_TRUE)

        out_sb = singles.tile([128, 56], F32)
        nc.vector.memset(out_sb, 0.0)
        zfa = out_sb[:, 0:24]     # col rb*3 + idx
        zfs = out_sb[:, 24:40]    # col rb*2 + idx
        zp = out_sb[:, 40:48]
        s_ = out_sb[:, 48:56]

        for rb in range(RB):
            r0 = rb * 128
            na = ACT_PER_BLOCK[rb]
            dve_chunks = list(range(na, NFC))
            for c in range(NFC):
                fdt = fd_pool.tile([128, FCH], F32, tag="fdt", name=f"fd_{rb}_{c}")
                j0 = c * FCH
                nc.tensor.matmul(
                    fdt[:, 0:512],
                    lhsT=afs_sb[0:KS, r0 : r0 + 128],
                    rhs=bfs_sb[0:KS, j0 : j0 + 512],
                    start=True,
                    stop=True,
                )
                nc.tensor.matmul(
                    fdt[:, 512:1024],
                    lhsT=afs_sb[64 : 64 + KS, r0 : r0 + 128],
                    rhs=bfs_sb[64 : 64 + KS, j0 + 512 : j0 + 1024],
                    start=True,
                    stop=True,
                    tile_position=(64, 0),
                )
                if c < na:
                    nc.scalar.activation(
                        out=fdt[:, :],
                        in_=fdt[:, :],
                        func=AF.Exp,
                        scale=1.0 / A_TRUE,
                        bias=bias_sb[:, 0:1],
                        accum_out=zfa[:, rb * 3 + c : rb * 3 + c + 1],
                    )
                else:
                    di = dve_chunks.index(c)
                    e16 = e16_pool.tile([128, FCH], I16, tag="e16")
                    nc.vector.tensor_scalar(
                        out=e16[:, :],
                        in0=fdt[:, :],
                        scalar1=0.0,
                        scalar2=None,
                        op0=OP.max,
                    )
                    zcol = zfs[:, rb * 2 + di : rb * 2 + di + 1]
                    if POOL_HALVE:
                        half = h_pool.tile([128, 512], BF16, tag="half")
                        nc.gpsimd.tensor_tensor(
                            out=half[:, :],
                            in0=e16[:, 0:512].bitcast(BF16),
                            in1=e16[:, 512:1024].bitcast(BF16),
                            op=OP.add,
                        )
                        scr = scr_pool.tile([128, 512], BF16, tag="scr")
                        nc.vector.tensor_scalar(
                            out=scr[:, :],
                            in0=half[:, :],
                            scalar1=1.0,
                            scalar2=0.0,
                            op0=OP.mult,
                            op1=OP.add,
                            accum_out=zcol,
                        )
                    else:
                        scr = scr_pool.tile([128, FCH], BF16, tag="scr")
                        nc.vector.tensor_scalar(
                            out=scr[:, :],
                            in0=e16[:, :].bitcast(BF16),
                            scalar1=1.0,
                            scalar2=0.0,
                            op0=OP.mult,
                            op1=OP.add,
                            accum_out=zcol,
                        )
            # ---- band ----
            fdbt = fdb_pool.tile([128, W], F32, tag="fdbt", name=f"fdb_{rb}")
            nc.tensor.matmul(
                fdbt[:, :],
                lhsT=afr_sb[0:KB, r0 : r0 + 128],
                rhs=bfb_sb[0:KB, rb * W : (rb + 1) * W],
                start=True,
                stop=True,
            )
            pb = 96 if rb % 2 == 0 else 64
            pdbt = pdb_pool.tile([128, W], F32, tag="pdbt", name=f"pdb_{rb}")
            nc.tensor.matmul(
                pdbt[:, :],
                lhsT=apt_sb[pb : pb + KP, r0 : r0 + 128],
                rhs=bpt_sb[pb : pb + KP, rb * W : (rb + 1) * W],
                start=True,
                stop=True,
                tile_position=(pb, 0),
            )
            ep = ep_pool.tile([128, W], BF16, tag="ep")
            nc.scalar.activation(
                out=ep, in_=pdbt[:, :], func=AF.Exp, accum_out=zp[:, rb : rb + 1]
            )
            sscr = ss_pool.tile([128, W], BF16, tag="sscr")
            nc.vector.scalar_tensor_tensor(
                out=sscr,
                in0=fdbt[:, :],
                scalar=1.0,
                in1=ep[:, :],
                op0=OP.mult,
                op1=OP.mult,
                accum_out=s_[:, rb : rb + 1],
            )

        nc.sync.dma_start(out=outp[:, :], in_=out_sb[:, :])
    return nc


def _morton(p, bits=10):
    q = np.minimum((p * (1 << bits)).astype(np.uint64), (1 << bits) - 1)
    code = np.zeros(len(p), np.uint64)
    for b in range(bits):
        for dim in range(3):
            code |= ((q[:, dim] >> np.uint64(b)) & np.uint64(1)) << np.uint64(3 * b + dim)
    return code


def _fp22(x):
    return (x.view(np.uint32) & np.uint32(0xFFFFFC00)).view(np.float32)


def _prep_batch(b, points, pointfea1, pointfea2, weights):
    perm = np.argsort(_morton(points[b]))
    p = points[b][perm]
    f1 = pointfea1[b][perm]
    f2 = pointfea2[b][perm]

    f1sq = (f1 * f1).sum(1)
    f2sq = (f2 * f2).sum(1)
    onesN = np.ones((N, 1), np.float32)
    a_fea = _fp22(np.concatenate([2.0 * f1, onesN, f1sq[:, None]], 1).astype(np.float32))
    b_fea = _fp22(np.concatenate([f2, -f2sq[:, None], -onesN], 1).astype(np.float32))

    c = np.float32(CSCALE)
    f1c = c * f1
    f2c = c * f2
    f1csq = (f1c * f1c).sum(1)
    f2csq = (f2c * f2c).sum(1)
    au = np.float32(A_USED)
    a_s = np.concatenate(
        [2.0 * au * f1c, au * onesN, (au * f1csq)[:, None], onesN], 1
    ).astype(ml_dtypes.bfloat16)
    b_s = np.concatenate(
        [f2c, -f2csq[:, None], -onesN, np.float32(BCONST) * onesN], 1
    ).astype(ml_dtypes.bfloat16)
    return p, a_fea, b_fea, a_s, b_s


def make_in_maps(points, pointfea1, pointfea2, weights):
    points = np.asarray(points, np.float32)
    pointfea1 = np.asarray(pointfea1, np.float32)
    pointfea2 = np.asarray(pointfea2, np.float32)
    weights = np.asarray(weights, np.float32)

    inv = np.float32(1.0 / (SIGMA * SIGMA))
    batch_data = [
        _prep_batch(b, points, pointfea1, pointfea2, weights) for b in range(B)
    ]
    in_maps = []
    for k in range(NCORES):
        b = k // CPB
        r0 = (k % CPB) * ROWS
        p, a_fea, b_fea, a_s, b_s = batch_data[b]
        bpt_band = np.empty((KP, RB * W), np.float32)
        bfb_band = np.empty((KB, RB * W), np.float32)
        apt_core = np.empty((KP, ROWS), np.float32)
        ones128 = np.ones((128, 1), np.float32)
        onesW = np.ones((W, 1), np.float32)
        for rb in range(RB):
            g0 = r0 + rb * 128
            s = min(max(g0 - PAD, 0), N - W)
            rows = p[g0 : g0 + 128]
            band = p[s : s + W]
            allp = np.concatenate([band, rows])
            ctr = (allp.min(0) + allp.max(0)) / 2
            pr = rows - ctr
            pb = band - ctr
            pr2 = (pr * pr).sum(1)
            pb2 = (pb * pb).sum(1)
            apt_core[:, rb * 128 : (rb + 1) * 128] = _fp22(
                np.concatenate([2.0 * pr * inv, ones128, (pr2 * inv)[:, None]], 1).astype(np.float32)
            ).T
            bpt_band[:, rb * W : (rb + 1) * W] = _fp22(
                np.concatenate([pb, -(pb2 * inv)[:, None], -onesW], 1).astype(np.float32)
            ).T
            bfb_band[:, rb * W : (rb + 1) * W] = b_fea[s : s + W].T
        ab = np.concatenate([a_s[r0 : r0 + ROWS].T, b_s.T], axis=1)
        in_maps.append(
            {
                "ab": np.ascontiguousarray(ab),
                "apt": apt_core,
                "bpt": bpt_band,
                "afr": np.ascontiguousarray(a_fea[r0 : r0 + ROWS].T),
                "bfb": bfb_band,
            }
        )
    return in_maps


def get_nc():
    if "nc" not in _CACHE:
        nc = _build()
        nc.finalize()
        _CACHE["nc"] = nc
    return _CACHE["nc"]


def combine_partials(parts, points, pointfea1, pointfea2, weights):
    """parts: [NCORES, 128, 56]. Host: Zf assembly, ln, ce sum, reg."""
    parts = np.asarray(parts, np.float64)
    weights = np.asarray(weights, np.float32)
    ce = np.zeros(B, np.float64)
    for k in range(NCORES):
        b = k // CPB
        r0 = (k % CPB) * ROWS
        pp = parts[k]
        zf = pp[:, 0:24].reshape(128, 8, 3).sum(2) + pp[:, 24:40].reshape(128, 8, 2).sum(2) / KAPPA
        zp = pp[:, 40:48]
        s = pp[:, 48:56]
        ce_rows = np.log(zf) - s / zp          # [128 part, 8 blocks]
        perm = _CACHE[f"perm{b}"]
        w = weights[b, :, 0][perm][r0 : r0 + ROWS].reshape(8, 128)  # [rb, p]
        ce[b] += (ce_rows.T * w).sum()
    f1 = np.asarray(pointfea1, np.float64)
    f2 = np.asarray(pointfea2, np.float64)
    reg = (f1[:, :, 3:] ** 2 + f2[:, :, 3:] ** 2).mean(2).mean(1)
    return ce.astype(np.float32), reg.astype(np.float32)


def kernel(points, pointfea1, pointfea2, weights):
    nc = get_nc()
    points = np.asarray(points, np.float32)
    for b in range(B):
        _CACHE[f"perm{b}"] = np.argsort(_morton(points[b]))
    in_maps = make_in_maps(points, pointfea1, pointfea2, weights)
    res = run_bass_kernel_spmd(nc, in_maps, core_ids=list(range(NCORES)))
    parts = np.stack([res.results[k]["partials"] for k in range(NCORES)])
    return combine_partials(parts, points, pointfea1, pointfea2, weights)


# revision 12
# speedup vs baseline: 1.3433x; 1.2153x over previous
"""Fused DeepFeatureLoss kernel for 8 Trainium2 NeuronCores (v5).

Reference computation (per batch b, N=4096 points, D=32 features):
    pd[i,j] = -||p_i - p_j||^2 / sigma^2          (points, sigma=0.005)
    fd[i,j] = -||f1_i - f2_j||^2
    ce[i]   = -sum_j softmax(pd)[i,j] * log_softmax(fd)[i,j]
    ce_loss[b]  = sum_i ce[i] * w[i]
    reg_loss[b] = mean_{i, c>=3} (f1[i,c]^2 + f2[i,c]^2)

Identity: ce[i] = ln(Zf_i) - S_i/Zp_i with
    Zf_i = sum_j exp(fd[i,j]);  Zp_i = sum_j exp(pd[i,j]);  S_i = sum_j exp(pd)*fd.

Per core (batch k//4, rows r0=1024*(k%4) .. +1024, 8 blocks of 128 rows):

fd path: the augmented K=35 bf16 matmul writes v = a*fd + b to PSUM in 1024-
col chunks (pool of 3 slots), a = 184 (bf16-exact; features pre-scaled by
sqrt((128/ln2)/184) on the host make it the exact Schraudolph constant),
b = 16256.  ACT chunks take exact exp via the activation's free affine and
row-sum on the ACT accumulator (zfa).  DVE chunks take the Schraudolph
route: tensor_scalar converts max(v,0) to int16 whose bits ARE bf16
exp(fd); Pool folds the bitcast tile twice (1024->512->256), DVE reduces
(zfs); the host divides by the staircase mean KAPPA.

band (Zp, S): each core's bfs columns are ROTATED so that SBUF chunk
c-hat = (global chunk - q) mod 4 (q = core%4).  The Gaussian band of block
rb then always occupies the same SBUF-local columns on every core: chunk 0
cols [128rb-128, 128rb+256) (wrapping into chunk 3 / chunk 1 at rb=0/7),
so one SPMD instruction stream serves all cores.  Wrapped columns are
spatially far (Morton ends), exp(pd) underflows to 0 and they contribute
nothing.  pd comes from a K=16 bf16 matmul over hi/lo-split recentered
points scaled by 256 (all channels bf16-exact or bf16 residuals; exact
products in the PE; validated ~1e-5): ep = exp(scl*m) on ACT with
scl = -(1/sigma^2)/256^2 = -0.6103515625 exactly, accum -> zp.
S is read off the SAME fd chunk PSUM (v-values): S_raw = sum ep*v via DVE
scalar_tensor_tensor accum; host: S = (S_raw - b*Zp)/a.  No separate band
fd matmul or operands.

Host: Zf = zfa + zfs/KAPPA, ce = w*(ln Zf - S/Zp); reg from f1/f2 (O(N)
postprocessing of device reductions, like the hint's final all-reduce).
"""

import math

import ml_dtypes
import numpy as np
from contextlib import ExitStack

import concourse.bacc as bacc
import concourse.bass as bass
import concourse.tile as tile
from concourse import mybir
from concourse.bass_utils import run_bass_kernel_spmd

SIGMA = 0.005
B, N, D = 2, 4096, 32
NCORES = 8
CPB = NCORES // B            # cores per batch = 4
ROWS = N // CPB              # rows per core = 1024
RB = ROWS // 128             # 128-row blocks per core = 8
NFC = 4                      # fd chunks per block
FCH = N // NFC               # 1024
W = 384                      # band width
KP = 16                      # hi/lo augmented K for points
KS = D + 3                   # augmented K for scaled fd = 35
F32 = mybir.dt.float32
BF16 = mybir.dt.bfloat16
I16 = mybir.dt.int16

A_TRUE = 128.0 / math.log(2.0)
A_USED = 184.0                        # bf16-exact
CSCALE = math.sqrt(A_TRUE / A_USED)
BCONST = 16256.0
KAPPA = 1.039720
C0 = 256.0                            # power-of-2 point scale
PSCL = -(1.0 / (SIGMA * SIGMA)) / (C0 * C0)   # -0.6103515625 exact

# per-block ACT chunk sets (c-hat indices).  Chunk 0 (and 3 at rb0, 1 at
# rb7) must stay on the DVE side: their PSUM v-values feed the band S.
ACT_SETS = [
    (1, 2),            # rb0  (band wraps into chunk 3)
    (1, 2, 3),
    (1, 2),
    (1, 2, 3),
    (1, 2),
    (1, 2, 3),
    (1, 2),
    (2, 3),            # rb7  (band wraps into chunk 1)
]

# band pieces per rb: list of (chat, col_off, width); ep columns follow in
# the same order.
def _band_pieces(rb):
    if rb == 0:
        return [(3, 896, 128), (0, 0, 256)]
    if rb == 7:
        return [(0, 768, 256), (1, 0, 128)]
    lo = 128 * rb - 128
    return [(0, lo, 384)]


_CACHE = {}


def _build():
    nc = bacc.Bacc(trn_type="TRN2")
    afs = nc.declare_dram_parameter("afs", [KS, ROWS], BF16, isOutput=False)
    bfse = nc.declare_dram_parameter("bfse", [KS, N // 2], BF16, isOutput=False)
    bfso = nc.declare_dram_parameter("bfso", [KS, N // 2], BF16, isOutput=False)
    apt = nc.declare_dram_parameter("apt", [KP, ROWS], BF16, isOutput=False)
    bpt = nc.declare_dram_parameter("bpt", [KP, RB * W], BF16, isOutput=False)
    outp = nc.declare_dram_parameter("partials", [128, 58], F32, isOutput=True)

    AF = mybir.ActivationFunctionType
    OP = mybir.AluOpType

    with ExitStack() as ctx:
        tc = ctx.enter_context(tile.TileContext(nc))
        singles = ctx.enter_context(tc.tile_pool(name="singles", bufs=1))
        fd_pool = ctx.enter_context(tc.tile_pool(name="fdp", bufs=3, space="PSUM"))
        pdb_pool = ctx.enter_context(tc.tile_pool(name="pdbp", bufs=2, space="PSUM"))
        e16_pool = ctx.enter_context(tc.tile_pool(name="e16p", bufs=2))
        h1_pool = ctx.enter_context(tc.tile_pool(name="h1p", bufs=2))
        h2_pool = ctx.enter_context(tc.tile_pool(name="h2p", bufs=2))
        scr_pool = ctx.enter_context(tc.tile_pool(name="scrp", bufs=2))
        ep_pool = ctx.enter_context(tc.tile_pool(name="epp", bufs=2))
        ss_pool = ctx.enter_context(tc.tile_pool(name="ssp", bufs=2))

        # ---- input loads ----
        afs_sb = singles.tile([128, ROWS], BF16)
        bfse_sb = singles.tile([KS, N // 2], BF16)
        bfso_sb = singles.tile([128, N // 2], BF16)
        # SP queue: group-0 operands (stationary block 0 first, then moving)
        nc.sync.dma_start(out=afs_sb[0:KS, 0:128], in_=afs[:, 0:128])
        nc.sync.dma_start(out=bfse_sb[:, 0:512], in_=bfse[:, 0:512])
        nc.sync.dma_start(out=afs_sb[0:KS, 128:1024], in_=afs[:, 128:1024])
        nc.sync.dma_start(out=bfse_sb[:, 512:2048], in_=bfse[:, 512:2048])
        # ACT queue: group-64 operands
        nc.scalar.dma_start(out=afs_sb[64 : 64 + KS, 0:128], in_=afs[:, 0:128])
        nc.scalar.dma_start(out=bfso_sb[64 : 64 + KS, 0:512], in_=bfso[:, 0:512])
        nc.scalar.dma_start(out=afs_sb[64 : 64 + KS, 128:1024], in_=afs[:, 128:1024])
        nc.scalar.dma_start(out=bfso_sb[64 : 64 + KS, 512:2048], in_=bfso[:, 512:2048])
        # Pool SWDGE: band point operands
        apt_sb = singles.tile([128, ROWS], BF16)
        bpt_sb = singles.tile([128, RB * W], BF16)
        nc.gpsimd.dma_start(out=apt_sb[96 : 96 + KP, :], in_=apt[:, :])
        nc.gpsimd.dma_start(out=bpt_sb[96 : 96 + KP, :], in_=bpt[:, :])
        nc.gpsimd.dma_start(out=apt_sb[64 : 64 + KP, :], in_=apt[:, :])
        nc.gpsimd.dma_start(out=bpt_sb[64 : 64 + KP, :], in_=bpt[:, :])

        bias_sb = singles.tile([128, 1], F32)
        nc.vector.memset(bias_sb, -BCONST / A_TRUE)

        out_sb = singles.tile([128, 58], F32)
        nc.vector.memset(out_sb, 0.0)
        zfa = out_sb[:, 0:24]     # rb*3 + idx
        zfs = out_sb[:, 24:40]    # rb*2 + idx
        zp = out_sb[:, 40:48]
        s_ = out_sb[:, 48:56]     # first band piece per rb
        s2 = out_sb[:, 56:58]     # second piece (rb0, rb7)

        for rb in range(RB):
            r0 = rb * 128
            act_set = ACT_SETS[rb]
            pieces = _band_pieces(rb)
            # ---- band pd matmul + ep, emitted first so ep is ready when
            # the chunk-0 v-values land ----
            pb = 96 if rb % 2 == 0 else 64
            pdbt = pdb_pool.tile([128, W], F32, tag="pdbt", name=f"pdb_{rb}")
            nc.tensor.matmul(
                pdbt[:, :],
                lhsT=apt_sb[pb : pb + KP, r0 : r0 + 128],
                rhs=bpt_sb[pb : pb + KP, rb * W : (rb + 1) * W],
                start=True,
                stop=True,
                tile_position=(pb, 0),
            )
            ep = ep_pool.tile([128, W], F32, tag="ep")
            nc.scalar.activation(
                out=ep,
                in_=pdbt[:, :],
                func=AF.Exp,
                scale=PSCL,
                accum_out=zp[:, rb : rb + 1],
            )
            # ---- fd chunks ----
            ndve = 0
            epoff = [0]
            for pc in pieces:
                epoff.append(epoff[-1] + pc[2])
            for ch in range(NFC):
                fdt = fd_pool.tile([128, FCH], F32, tag="fdt", name=f"fd_{rb}_{ch}")
                nc.tensor.matmul(
                    fdt[:, 0:512],
                    lhsT=afs_sb[0:KS, r0 : r0 + 128],
                    rhs=bfse_sb[0:KS, ch * 512 : (ch + 1) * 512],
                    start=True,
                    stop=True,
                )
                nc.tensor.matmul(
                    fdt[:, 512:1024],
                    lhsT=afs_sb[64 : 64 + KS, r0 : r0 + 128],
                    rhs=bfso_sb[64 : 64 + KS, ch * 512 : (ch + 1) * 512],
                    start=True,
                    stop=True,
                    tile_position=(64, 0),
                )
                if ch in act_set:
                    idx = act_set.index(ch)
                    nc.scalar.activation(
                        out=fdt[:, :],
                        in_=fdt[:, :],
                        func=AF.Exp,
                        scale=1.0 / A_TRUE,
                        bias=bias_sb[:, 0:1],
                        accum_out=zfa[:, rb * 3 + idx : rb * 3 + idx + 1],
                    )
                else:
                    e16 = e16_pool.tile([128, FCH], I16, tag="e16")
                    nc.vector.tensor_scalar(
                        out=e16[:, :],
                        in0=fdt[:, :],
                        scalar1=0.0,
                        scalar2=None,
                        op0=OP.max,
                    )
                    h1 = h1_pool.tile([128, 512], BF16, tag="h1")
                    nc.gpsimd.tensor_tensor(
                        out=h1[:, :],
                        in0=e16[:, 0:512].bitcast(BF16),
                        in1=e16[:, 512:1024].bitcast(BF16),
                        op=OP.add,
                    )
                    h2 = h2_pool.tile([128, 256], BF16, tag="h2")
                    nc.gpsimd.tensor_tensor(
                        out=h2[:, :],
                        in0=h1[:, 0:256],
                        in1=h1[:, 256:512],
                        op=OP.add,
                    )
                    scr = scr_pool.tile([128, 256], BF16, tag="scr")
                    nc.vector.tensor_scalar(
                        out=scr[:, :],
                        in0=h2[:, :],
                        scalar1=1.0,
                        scalar2=0.0,
                        op0=OP.mult,
                        op1=OP.add,
                        accum_out=zfs[:, rb * 2 + ndve : rb * 2 + ndve + 1],
                    )
                    ndve += 1
                # band S pieces living in this chunk: read v from PSUM
                for pi, (pch, off, wd) in enumerate(pieces):
                    if pch != ch:
                        continue
                    scol = (
                        s_[:, rb : rb + 1]
                        if pi == 0
                        else s2[:, (0 if rb == 0 else 1) : (1 if rb == 0 else 2)]
                    )
                    sscr = ss_pool.tile([128, wd], BF16, tag="sscr", name=f"ss_{rb}_{pi}")
                    nc.vector.scalar_tensor_tensor(
                        out=sscr,
                        in0=fdt[:, off : off + wd],
                        scalar=1.0,
                        in1=ep[:, epoff[pi] : epoff[pi] + wd],
                        op0=OP.mult,
                        op1=OP.mult,
                        accum_out=scol,
                    )

        nc.sync.dma_start(out=outp[:, :], in_=out_sb[:, :])
    return nc


def _morton(p, bits=10):
    q = np.minimum((p * (1 << bits)).astype(np.uint64), (1 << bits) - 1)
    code = np.zeros(len(p), np.uint64)
    for b in range(bits):
        for dim in range(3):
            code |= ((q[:, dim] >> np.uint64(b)) & np.uint64(1)) << np.uint64(3 * b + dim)
    return code


def _bf(x):
    return np.asarray(x, np.float32).astype(ml_dtypes.bfloat16)


def _bff(x):
    return _bf(x).astype(np.float32)


def _prep_batch(b, points, pointfea1, pointfea2):
    perm = np.argsort(_morton(points[b]))
    p = points[b][perm]
    f1 = pointfea1[b][perm]
    f2 = pointfea2[b][perm]

    c = np.float32(CSCALE)
    f1c = c * f1
    f2c = c * f2
    f1csq = (f1c * f1c).sum(1)
    f2csq = (f2c * f2c).sum(1)
    au = np.float32(A_USED)
    onesN = np.ones((N, 1), np.float32)
    a_s = np.concatenate(
        [2.0 * au * f1c, au * onesN, (au * f1csq)[:, None], onesN], 1
    ).astype(ml_dtypes.bfloat16)
    b_s = np.concatenate(
        [f2c, -f2csq[:, None], -onesN, np.float32(BCONST) * onesN], 1
    ).astype(ml_dtypes.bfloat16)
    return p, a_s, b_s


def make_in_maps(points, pointfea1, pointfea2, weights):
    points = np.asarray(points, np.float32)
    pointfea1 = np.asarray(pointfea1, np.float32)
    pointfea2 = np.asarray(pointfea2, np.float32)

    batch_data = [_prep_batch(b, points, pointfea1, pointfea2) for b in range(B)]
    in_maps = []
    for k in range(NCORES):
        b = k // CPB
        q = k % CPB
        r0 = q * ROWS
        p, a_s, b_s = batch_data[b]
        # rotated chunk layout: SBUF chunk chat holds global chunk (chat+q)%4
        bT = b_s.T  # [KS, N]
        bfse = np.empty((KS, N // 2), ml_dtypes.bfloat16)
        bfso = np.empty((KS, N // 2), ml_dtypes.bfloat16)
        for chat in range(NFC):
            g = ((chat + q) % NFC) * FCH
            bfse[:, chat * 512 : (chat + 1) * 512] = bT[:, g : g + 512]
            bfso[:, chat * 512 : (chat + 1) * 512] = bT[:, g + 512 : g + 1024]
        # band points per block, in SBUF-local band order
        apt_core = np.empty((KP, ROWS), ml_dtypes.bfloat16)
        bpt_band = np.empty((KP, RB * W), ml_dtypes.bfloat16)
        for rb in range(RB):
            g0 = r0 + rb * 128
            rows = p[g0 : g0 + 128]
            cols_idx = []
            for chat, off, wd in _band_pieces(rb):
                gch = ((chat + q) % NFC) * FCH
                cols_idx.extend(range(gch + off, gch + off + wd))
            band = p[np.array(cols_idx)]
            allp = np.concatenate([band, rows])
            ctr = (allp.min(0) + allp.max(0)) / 2
            qr = ((rows - ctr) * C0).astype(np.float32)
            qb = ((band - ctr) * C0).astype(np.float32)
            qrh = _bff(qr); qrl = _bf(qr - qrh)
            qbh = _bff(qb); qbl = _bf(qb - qbh)
            sqr = ((qr.astype(np.float64) ** 2).sum(1)).astype(np.float32)
            sqb = ((qb.astype(np.float64) ** 2).sum(1)).astype(np.float32)
            sqrh = _bff(sqr); sqrl = _bf(sqr - sqrh)
            sqbh = _bff(sqb); sqbl = _bf(sqb - sqbh)
            o_r = np.ones((128, 1), np.float32)
            o_b = np.ones((W, 1), np.float32)
            qrh = qrh.astype(np.float32); qbh = qbh.astype(np.float32)
            A = np.concatenate(
                [-2 * qrh, -2 * qrl.astype(np.float32), -2 * qrh, -2 * qrl.astype(np.float32),
                 sqrh[:, None], sqrl.astype(np.float32)[:, None], o_r, o_r], 1)
            Bm = np.concatenate(
                [qbh, qbh, qbl.astype(np.float32), qbl.astype(np.float32),
                 o_b, o_b, sqbh[:, None], sqbl.astype(np.float32)[:, None]], 1)
            apt_core[:, rb * 128 : (rb + 1) * 128] = _bf(A).T
            bpt_band[:, rb * W : (rb + 1) * W] = _bf(Bm).T
        in_maps.append(
            {
                "afs": np.ascontiguousarray(a_s[r0 : r0 + ROWS].T),
                "bfse": np.ascontiguousarray(bfse),
                "bfso": np.ascontiguousarray(bfso),
                "apt": np.ascontiguousarray(apt_core),
                "bpt": np.ascontiguousarray(bpt_band),
            }
        )
    return in_maps


def get_nc():
    if "nc" not in _CACHE:
        nc = _build()
        nc.finalize()
        _CACHE["nc"] = nc
    return _CACHE["nc"]


def combine_partials(parts, points, pointfea1, pointfea2, weights):
    """parts: [NCORES, 128, 58]."""
    parts = np.asarray(parts, np.float64)
    weights = np.asarray(weights, np.float32)
    ce = np.zeros(B, np.float64)
    for k in range(NCORES):
        b = k // CPB
        r0 = (k % CPB) * ROWS
        pp = parts[k]
        zf = pp[:, 0:24].reshape(128, 8, 3).sum(2) + pp[:, 24:40].reshape(128, 8, 2).sum(2) / KAPPA
        zp = pp[:, 40:48]
        s_raw = pp[:, 48:56].copy()
        s_raw[:, 0] += pp[:, 56]
        s_raw[:, 7] += pp[:, 57]
        s = (s_raw - BCONST * zp) / A_TRUE
        ce_rows = np.log(zf) - s / zp          # [128 part, 8 blocks]
        perm = _CACHE[f"perm{b}"]
        w = weights[b, :, 0][perm][r0 : r0 + ROWS].reshape(8, 128)  # [rb, p]
        ce[b] += (ce_rows.T * w).sum()
    f1 = np.asarray(pointfea1, np.float64)
    f2 = np.asarray(pointfea2, np.float64)
    reg = (f1[:, :, 3:] ** 2 + f2[:, :, 3:] ** 2).mean(2).mean(1)
    return ce.astype(np.float32), reg.astype(np.float32)


def kernel(points, pointfea1, pointfea2, weights):
    nc = get_nc()
    points = np.asarray(points, np.float32)
    for b in range(B):
        _CACHE[f"perm{b}"] = np.argsort(_morton(points[b]))
    in_maps = make_in_maps(points, pointfea1, pointfea2, weights)
    res = run_bass_kernel_spmd(nc, in_maps, core_ids=list(range(NCORES)))
    parts = np.stack([res.results[k]["partials"] for k in range(NCORES)])
    return combine_partials(parts, points, pointfea1, pointfea2, weights)
